# revision 5
# baseline (speedup 1.0000x reference)
"""AlphaKnotLoss on 8 TRN2 NeuronCores (Bass/Tile, SPMD data-parallel).

Reference computation (B=4096 graphs x 512 nodes x A=10 actions):
  loss_val    = mean((values - target_vals)^2)                  over B
  per graph g: Z[g]   = sum_{n in g, a} exp(logits[n,a])
               Lin[g] = sum_{n in g, a} target_probs[n,a]*logits[n,a]
               V[g]   = sum_{n in g, a} target_probs[n,a]
               lp[g]  = (log(Z[g]+eps) - Lin[g]) / (V[g]+eps)
  out = loss_val + mean(lp)

Sharding: data-parallel over graphs. Each of the 8 cores owns 512 whole
graphs = 262144 nodes. Per-core layout: the (262144, 10) node-major shard
is viewed as (128, 20480) so partition p holds 4 whole graphs
(4 x 512 nodes x 10 actions = 20480 contiguous floats). A free-axis
reduction over a 5120-wide block is exactly one graph's sum.

Per tile (128 x 5120): ACT does exp with fused accumulate (Z), DVE does
fused multiply+reduce (Lin) and a plain reduce (V). Epilogue computes
per-graph losses, reduces 512 graphs to per-partition partials, uses one
matmul against a ones-vector for the cross-partition sum, all-reduces the
2-element partial vector across cores, and every core writes the final
scalar.
"""

import numpy as np

B = 4096
NPG = 512
N = B * NPG
A = 10
EPS = 1e-9
M = 8  # cores

NC_NODES = N // M          # 262144 nodes per core
NC_GRAPHS = B // M         # 512 graphs per core
P = 128                    # SBUF partitions
FREE = NC_NODES * A // P   # 20480 f32 per partition
GPP = NC_GRAPHS // P       # 4 graphs per partition
GFREE = NPG * A            # 5120 f32 per graph

NT = 4                     # free-dim tiles (one graph-column per tile)
F = FREE // NT             # 5120

# "allreduce": final sums combined on-device, every core outputs the scalar.
# "partials":  each core outputs its (1,2) partial sums; host combines.
MODE = "allreduce"

_CACHE = {}


def _build(mode):
    import concourse.bacc as bacc
    import concourse.mybir as mybir
    import concourse.tile as tile

    f32 = mybir.dt.float32
    Alu = mybir.AluOpType
    Act = mybir.ActivationFunctionType
    AX = mybir.AxisListType.X

    nc = bacc.Bacc("TRN2", target_bir_lowering=False, debug=False,
                   num_devices=M)

    logits = nc.dram_tensor("logits", [P, FREE], f32, kind="ExternalInput")
    probs = nc.dram_tensor("probs", [P, FREE], f32, kind="ExternalInput")
    vals = nc.dram_tensor("vals", [P, GPP], f32, kind="ExternalInput")
    tvals = nc.dram_tensor("tvals", [P, GPP], f32, kind="ExternalInput")
    out = nc.dram_tensor("out", [1, 8], f32, kind="ExternalOutput")

    with tile.TileContext(nc) as tc:
        with (
            tc.tile_pool(name="io", bufs=2) as iop,
            tc.tile_pool(name="work", bufs=2) as wp,
            tc.tile_pool(name="stats", bufs=1) as sp,
            tc.tile_pool(name="psum", bufs=1, space="PSUM") as pp,
            tc.tile_pool(name="dram", bufs=1, space="DRAM") as dp,
        ):
            Z = sp.tile([P, NT], f32)
            Lin = sp.tile([P, NT], f32)
            V = sp.tile([P, NT], f32)

            for j in range(NT):
                lt = iop.tile([P, F], f32, tag="lt")
                pt = iop.tile([P, F], f32, tag="pt")
                et = wp.tile([P, F], f32, tag="et")
                prod = wp.tile([P, F], f32, tag="prod")
                nc.sync.dma_start(lt[:, :], logits[:, j * F:(j + 1) * F])
                nc.sync.dma_start(pt[:, :], probs[:, j * F:(j + 1) * F])
                nc.scalar.activation(et[:, :], lt[:, :], Act.Exp,
                                     accum_out=Z[:, j:j + 1])
                nc.vector.reduce_sum(V[:, j:j + 1], pt[:, :], axis=AX)
                nc.vector.tensor_mul(prod[:, :], pt[:, :], lt[:, :])
                nc.vector.reduce_sum(Lin[:, j:j + 1], prod[:, :], axis=AX)

            # per-graph policy loss: (ln(Z+eps) - Lin) / (V+eps)
            zp = sp.tile([P, NT], f32)
            nc.vector.tensor_scalar_add(zp[:, :], Z[:, :], EPS)
            logz = sp.tile([P, NT], f32)
            nc.scalar.activation(logz[:, :], zp[:, :], Act.Ln)
            num = sp.tile([P, NT], f32)
            nc.vector.tensor_sub(num[:, :], logz[:, :], Lin[:, :])
            den = sp.tile([P, NT], f32)
            nc.vector.tensor_scalar_add(den[:, :], V[:, :], EPS)
            rec = sp.tile([P, NT], f32)
            nc.vector.reciprocal(rec[:, :], den[:, :])
            lp = sp.tile([P, NT], f32)
            nc.vector.tensor_mul(lp[:, :], num[:, :], rec[:, :])

            # S[:,0] = per-partition policy sum, S[:,1] = value-sq sum
            S = sp.tile([P, 8], f32)
            nc.gpsimd.memset(S[:, :], 0.0)
            nc.vector.reduce_sum(S[:, 0:1], lp[:, :], axis=AX)

            vt = sp.tile([P, GPP], f32)
            tt = sp.tile([P, GPP], f32)
            nc.sync.dma_start(vt[:, :], vals[:, :])
            nc.sync.dma_start(tt[:, :], tvals[:, :])
            d = sp.tile([P, GPP], f32)
            nc.vector.tensor_sub(d[:, :], vt[:, :], tt[:, :])
            d2 = sp.tile([P, GPP], f32)
            nc.vector.tensor_mul(d2[:, :], d[:, :], d[:, :])
            nc.vector.reduce_sum(S[:, 1:2], d2[:, :], axis=AX)

            # cross-partition sum via matmul with a ones vector
            ones = sp.tile([P, 1], f32)
            nc.gpsimd.memset(ones[:, :], 1.0)
            ps = pp.tile([1, 8], f32)
            nc.tensor.matmul(ps[:, :], ones[:, :], S[:, :],
                             start=True, stop=True)
            red = sp.tile([1, 8], f32)
            nc.vector.tensor_copy(red[:, :], ps[:, :])

            if mode == "allreduce":
                cin = dp.tile([1, 8], f32)
                cout = dp.tile([1, 8], f32)
                nc.sync.dma_start(cin[:, :], red[:, :])
                nc.gpsimd.collective_compute(
                    "AllReduce", mybir.AluOpType.add,
                    replica_groups=[list(range(M))],
                    ins=[cin[:, :].opt()],
                    outs=[cout[:, :].opt()])
                red2 = sp.tile([1, 8], f32)
                nc.sync.dma_start(red2[:, :], cout[:, :])
                # out = (sum_policy + sum_val) / B
                dummy = sp.tile([1, 2], f32)
                fin = sp.tile([1, 8], f32)
                nc.gpsimd.memset(fin[:, :], 0.0)
                nc.scalar.activation(dummy[:, :], red2[:, 0:2], Act.Copy,
                                     scale=1.0 / B, accum_out=fin[:, 0:1])
                nc.sync.dma_start(out[:, :], fin[:, :])
            else:
                nc.sync.dma_start(out[:, :], red[:, :])

    nc.compile()
    return nc


def _get(mode):
    if mode not in _CACHE:
        _CACHE[mode] = _build(mode)
    return _CACHE[mode]


def _make_in_maps(logits, values, target_probs, target_vals):
    in_maps = []
    lg = logits.reshape(M, P, FREE)
    pg = target_probs.reshape(M, P, FREE)
    vg = values.reshape(M, P, GPP)
    tg = target_vals.reshape(M, P, GPP)
    for c in range(M):
        in_maps.append({
            "logits": np.ascontiguousarray(lg[c]),
            "probs": np.ascontiguousarray(pg[c]),
            "vals": np.ascontiguousarray(vg[c]),
            "tvals": np.ascontiguousarray(tg[c]),
        })
    return in_maps


def _finalize(mode, results):
    if mode == "allreduce":
        return np.float32(results[0]["out"][0, 0])
    parts = np.stack([r["out"][0] for r in results])  # (M, 2)
    tot = parts.sum(axis=0, dtype=np.float64)
    return np.float32((tot[0] + tot[1]) / B)


def kernel(logits, values, target_probs, target_vals, batch_counts):
    from concourse import bass_utils

    logits = np.asarray(logits, dtype=np.float32)
    values = np.asarray(values, dtype=np.float32)
    target_probs = np.asarray(target_probs, dtype=np.float32)
    target_vals = np.asarray(target_vals, dtype=np.float32)
    batch_counts = np.asarray(batch_counts)

    if not (batch_counts.shape == (B,) and np.all(batch_counts == NPG)):
        # Non-uniform segments never occur for this problem's inputs;
        # numpy fallback keeps the contract total.
        return _kernel_numpy(logits, values, target_probs, target_vals,
                             batch_counts)

    nc = _get(MODE)
    in_maps = _make_in_maps(logits, values, target_probs, target_vals)
    res = bass_utils.run_bass_kernel_spmd(nc, in_maps, core_ids=list(range(M)))
    return _finalize(MODE, res.results)


def _kernel_numpy(logits, values, target_probs, target_vals, batch_counts):
    counts = batch_counts.astype(np.int64)
    b = counts.shape[0]
    idx = np.repeat(np.arange(b), counts)
    loss_val = np.mean((values - target_vals) ** 2, dtype=np.float32)
    probs_sum = target_probs.sum(axis=1)
    lin = (target_probs * logits).sum(axis=1)
    ex = np.exp(logits).sum(axis=1)
    vc = np.zeros(b, np.float32)
    lg = np.zeros(b, np.float32)
    zg = np.zeros(b, np.float32)
    np.add.at(vc, idx, probs_sum)
    np.add.at(lg, idx, lin)
    np.add.at(zg, idx, ex)
    lp = (np.log(zg + EPS) - lg) / (vc + EPS)
    return np.float32(loss_val + lp.mean())


# revision 6
# speedup vs baseline: 2.1667x; 2.1667x over previous
"""AlphaKnotLoss on 8 TRN2 NeuronCores (Bass/Tile, SPMD data-parallel).

Reference computation (B=4096 graphs x 512 nodes x A=10 actions):
  loss_val    = mean((values - target_vals)^2)                  over B
  per graph g: Z[g]   = sum_{n in g, a} exp(logits[n,a])
               Lin[g] = sum_{n in g, a} target_probs[n,a]*logits[n,a]
               V[g]   = sum_{n in g, a} target_probs[n,a]
               lp[g]  = (log(Z[g]+eps) - Lin[g]) / (V[g]+eps)
  out = loss_val + mean(lp)

Sharding: data-parallel over graphs. Each of the 8 cores owns 512 whole
graphs = 262144 nodes. Per-core layout: the (262144, 10) node-major shard
is viewed as (128, 20480) so partition p holds 4 whole graphs
(4 x 512 nodes x 10 actions = 20480 contiguous floats); a graph is a
5120-wide contiguous block of the free axis, split across NT tiles.

Per tile (128 x F): ACT does exp with fused accumulate (Z) and a
copy-accumulate (V); DVE does one fused multiply+reduce via
scalar_tensor_tensor (Lin). The loop is DMA-bound (~56us of HBM traffic
per core); ACT ~37us and DVE ~22us hide underneath it.

Epilogue: per-graph losses on (128, GPP) stats, one PE matmul against a
ones vector for the cross-partition sum, then either a tiny AllReduce
(mode "allreduce": every core returns the final scalar) or per-core
partial sums combined on the host (mode "partials", default: avoids the
collective's mesh-entry barrier, which couples every core to the
slowest/last-started core).
"""

import numpy as np

B = 4096
NPG = 512
N = B * NPG
A = 10
EPS = 1e-9
M = 8  # cores

NC_NODES = N // M          # 262144 nodes per core
NC_GRAPHS = B // M         # 512 graphs per core
P = 128                    # SBUF partitions
FREE = NC_NODES * A // P   # 20480 f32 per partition
GPP = NC_GRAPHS // P       # 4 graphs per partition
GFREE = NPG * A            # 5120 f32 per graph

NT = 8                     # free-dim tiles
F = FREE // NT             # 2560
TPG = GFREE // F           # tiles per graph (2)

IO_BUFS = 3
WORK_BUFS = 2

MODE = "partials"

_CACHE = {}


def _build(mode):
    import concourse.bacc as bacc
    import concourse.mybir as mybir
    import concourse.tile as tile

    f32 = mybir.dt.float32
    Alu = mybir.AluOpType
    Act = mybir.ActivationFunctionType
    AX = mybir.AxisListType.X

    nc = bacc.Bacc("TRN2", target_bir_lowering=False, debug=False,
                   num_devices=M)

    logits = nc.dram_tensor("logits", [P, FREE], f32, kind="ExternalInput")
    probs = nc.dram_tensor("probs", [P, FREE], f32, kind="ExternalInput")
    vals = nc.dram_tensor("vals", [P, GPP], f32, kind="ExternalInput")
    tvals = nc.dram_tensor("tvals", [P, GPP], f32, kind="ExternalInput")
    out = nc.dram_tensor("out", [1, 8], f32, kind="ExternalOutput")

    with tile.TileContext(nc) as tc:
        with (
            tc.tile_pool(name="io", bufs=IO_BUFS) as iop,
            tc.tile_pool(name="work", bufs=WORK_BUFS) as wp,
            tc.tile_pool(name="stats", bufs=1) as sp,
            tc.tile_pool(name="psum", bufs=1, space="PSUM") as pp,
            tc.tile_pool(name="dram", bufs=1, space="DRAM") as dp,
        ):
            Z = sp.tile([P, NT], f32)
            Lin = sp.tile([P, NT], f32)
            V = sp.tile([P, NT], f32)

            for j in range(NT):
                lt = iop.tile([P, F], f32, tag="lt")
                pt = iop.tile([P, F], f32, tag="pt")
                et = wp.tile([P, F], f32, tag="et")
                cp = wp.tile([P, F], f32, tag="cp")
                prod = wp.tile([P, F], f32, tag="prod")
                nc.sync.dma_start(lt[:, :], logits[:, j * F:(j + 1) * F])
                nc.sync.dma_start(pt[:, :], probs[:, j * F:(j + 1) * F])
                nc.scalar.activation(et[:, :], lt[:, :], Act.Exp,
                                     accum_out=Z[:, j:j + 1])
                nc.scalar.activation(cp[:, :], pt[:, :], Act.Copy,
                                     accum_out=V[:, j:j + 1])
                nc.vector.scalar_tensor_tensor(
                    out=prod[:, :], in0=lt[:, :], scalar=1.0, in1=pt[:, :],
                    op0=Alu.mult, op1=Alu.mult,
                    accum_out=Lin[:, j:j + 1])

            # pair-reduce NT tile-partials down to GPP per-graph sums
            Zg = sp.tile([P, GPP], f32)
            Lg = sp.tile([P, GPP], f32)
            Vg = sp.tile([P, GPP], f32)
            for src, dst in ((Z, Zg), (Lin, Lg), (V, Vg)):
                nc.vector.reduce_sum(
                    dst[:, :],
                    src[:, :].rearrange("p (g t) -> p g t", t=TPG),
                    axis=AX)

            # per-graph policy loss: (ln(Z+eps) - Lin) / (V+eps)
            zp = sp.tile([P, GPP], f32)
            nc.vector.tensor_scalar_add(zp[:, :], Zg[:, :], EPS)
            logz = sp.tile([P, GPP], f32)
            nc.scalar.activation(logz[:, :], zp[:, :], Act.Ln)
            num = sp.tile([P, GPP], f32)
            nc.vector.tensor_sub(num[:, :], logz[:, :], Lg[:, :])
            den = sp.tile([P, GPP], f32)
            nc.vector.tensor_scalar_add(den[:, :], Vg[:, :], EPS)
            rec = sp.tile([P, GPP], f32)
            nc.vector.reciprocal(rec[:, :], den[:, :])

            # S[:,0] = per-partition policy sum, S[:,1] = value-sq sum
            S = sp.tile([P, 8], f32)
            nc.gpsimd.memset(S[:, :], 0.0)
            lp = sp.tile([P, GPP], f32)
            nc.vector.scalar_tensor_tensor(
                out=lp[:, :], in0=num[:, :], scalar=1.0, in1=rec[:, :],
                op0=Alu.mult, op1=Alu.mult, accum_out=S[:, 0:1])

            vt = sp.tile([P, GPP], f32)
            tt = sp.tile([P, GPP], f32)
            nc.sync.dma_start(vt[:, :], vals[:, :])
            nc.sync.dma_start(tt[:, :], tvals[:, :])
            d = sp.tile([P, GPP], f32)
            nc.vector.tensor_sub(d[:, :], vt[:, :], tt[:, :])
            d2 = sp.tile([P, GPP], f32)
            nc.vector.scalar_tensor_tensor(
                out=d2[:, :], in0=d[:, :], scalar=1.0, in1=d[:, :],
                op0=Alu.mult, op1=Alu.mult, accum_out=S[:, 1:2])

            # cross-partition sum via matmul with a ones vector
            ones = sp.tile([P, 1], f32)
            nc.gpsimd.memset(ones[:, :], 1.0)
            ps = pp.tile([1, 8], f32)
            nc.tensor.matmul(ps[:, :], ones[:, :], S[:, :],
                             start=True, stop=True)
            red = sp.tile([1, 8], f32)
            nc.vector.tensor_copy(red[:, :], ps[:, :])

            if mode == "allreduce":
                cin = dp.tile([1, 8], f32)
                cout = dp.tile([1, 8], f32)
                nc.sync.dma_start(cin[:, :], red[:, :])
                nc.gpsimd.collective_compute(
                    "AllReduce", Alu.add,
                    replica_groups=[list(range(M))],
                    ins=[cin[:, :].opt()],
                    outs=[cout[:, :].opt()])
                red2 = sp.tile([1, 8], f32)
                nc.sync.dma_start(red2[:, :], cout[:, :])
                # out = (sum_policy + sum_val) / B
                dummy = sp.tile([1, 2], f32)
                fin = sp.tile([1, 8], f32)
                nc.gpsimd.memset(fin[:, :], 0.0)
                nc.scalar.activation(dummy[:, :], red2[:, 0:2], Act.Copy,
                                     scale=1.0 / B, accum_out=fin[:, 0:1])
                nc.sync.dma_start(out[:, :], fin[:, :])
            else:
                nc.sync.dma_start(out[:, :], red[:, :])

    nc.compile()
    return nc


def _get(mode):
    if mode not in _CACHE:
        _CACHE[mode] = _build(mode)
    return _CACHE[mode]


def _make_in_maps(logits, values, target_probs, target_vals):
    in_maps = []
    lg = logits.reshape(M, P, FREE)
    pg = target_probs.reshape(M, P, FREE)
    vg = values.reshape(M, P, GPP)
    tg = target_vals.reshape(M, P, GPP)
    for c in range(M):
        in_maps.append({
            "logits": np.ascontiguousarray(lg[c]),
            "probs": np.ascontiguousarray(pg[c]),
            "vals": np.ascontiguousarray(vg[c]),
            "tvals": np.ascontiguousarray(tg[c]),
        })
    return in_maps


def _finalize(mode, results):
    if mode == "allreduce":
        return np.float32(results[0]["out"][0, 0])
    parts = np.stack([r["out"][0] for r in results])  # (M, 8)
    tot = parts.sum(axis=0, dtype=np.float64)
    return np.float32((tot[0] + tot[1]) / B)


def kernel(logits, values, target_probs, target_vals, batch_counts):
    from concourse import bass_utils

    logits = np.asarray(logits, dtype=np.float32)
    values = np.asarray(values, dtype=np.float32)
    target_probs = np.asarray(target_probs, dtype=np.float32)
    target_vals = np.asarray(target_vals, dtype=np.float32)
    batch_counts = np.asarray(batch_counts)

    if not (batch_counts.shape == (B,) and np.all(batch_counts == NPG)):
        # Non-uniform segments never occur for this problem's inputs;
        # numpy fallback keeps the contract total.
        return _kernel_numpy(logits, values, target_probs, target_vals,
                             batch_counts)

    nc = _get(MODE)
    in_maps = _make_in_maps(logits, values, target_probs, target_vals)
    res = bass_utils.run_bass_kernel_spmd(nc, in_maps, core_ids=list(range(M)))
    return _finalize(MODE, res.results)


def _kernel_numpy(logits, values, target_probs, target_vals, batch_counts):
    counts = batch_counts.astype(np.int64)
    b = counts.shape[0]
    idx = np.repeat(np.arange(b), counts)
    loss_val = np.mean((values - target_vals) ** 2, dtype=np.float32)
    probs_sum = target_probs.sum(axis=1)
    lin = (target_probs * logits).sum(axis=1)
    ex = np.exp(logits).sum(axis=1)
    vc = np.zeros(b, np.float32)
    lg = np.zeros(b, np.float32)
    zg = np.zeros(b, np.float32)
    np.add.at(vc, idx, probs_sum)
    np.add.at(lg, idx, lin)
    np.add.at(zg, idx, ex)
    lp = (np.log(zg + EPS) - lg) / (vc + EPS)
    return np.float32(loss_val + lp.mean())
